# revision 25
# baseline (speedup 1.0000x reference)
"""Trainium2 Bass kernel for nn_LowpassDetector.

Computes: power = re^2 + im^2, 5-tap FIR (b), order-4 IIR recurrence (a)
along time, for signal [2, T=16384, B=2048] -> y [T, B].

Strategy: the FIR+IIR cascade is LTI with all poles at radius <= 0.758,
so the combined impulse response decays below 1e-15 within 128 taps.
The whole filter is therefore exactly (to fp32) a block-Toeplitz matmul:
  y_blk[b] = T0 @ x_blk[b] + T1 @ x_blk[b-1]     (b >= 1)
  y_blk[0] = L0 @ x_blk[0]
with L0 the exact 128x128 operator of the reference recurrence
(including its "first 5 samples pass through" initial condition), built
on the host in float64. Channels (2048) are sharded 256 per core across
8 cores; time blocks of 128 map to the TensorEngine contraction dim.

v6 design (what measurement drove what):
- v3 was DMA-bound at the fp32 roofline (50.5 MB/core at ~310 GB/s).
  The rel-err budget (2e-2) has huge headroom (v3: 3.6e-4), so all I/O
  is fp16: host pre-packs the signal into the per-superbatch SBUF tile
  layout [NSB, 128, 2{re,im}, SBW*C] fp16 (one contiguous 1 MB DMA per
  superbatch, 8 KB/partition lines); y returns as fp16 tiles and is
  cast/scattered on the host. 24 MB/core total.
- Per-packet DMA rate is at line rate (26.5 GB/s/engine) but engines
  idled ~40% in multi-us gaps: pipeline stalls, not transfer speed.
  Root cause was head-of-line blocking in the in-order engine queues
  (e.g. DVE's PSUM drain for superbatch s waits on PE mid-queue and
  blocks superbatch s+1's square behind it). So the issue order is
  software-pipelined: drains and the output DMA of superbatch s-1 are
  issued at the top of iteration s; no instruction ever waits on a
  dependency from its own superbatch while blocking the next one.
- Engine balance per superbatch (~4.4 us DMA pace): ACT = 9C square +
  ps1 drain; DVE = 3C square (not in-place: in-place tensor_mul missed
  the fp16 2x path) + power add (fp16 2x) + ps0 drain + margin copy
  (bitcast to uint32 to halve element count); POOL (no PSUM port) =
  4C square. PE = 8 matmuls (t0 grouped then t1, 2 weight loads).
- PSUM rules learned the hard way: a matmul output region must not
  straddle a 2 KB bank boundary, and within one bank accumulation
  groups must not interleave (start=True clears the whole bank's
  has_written bits) -- each half-bank holds one group, opened once,
  closed once; the margin copy keeps block 0's T1 matmul in the same
  group shape as blocks 1-3.
"""

import sys
from contextlib import ExitStack

import numpy as np

for _p in ("/opt/trn_rl_repo",):
    if _p not in sys.path:
        sys.path.insert(0, _p)

import concourse.bass as bass  # noqa: E402
import concourse.tile as tile  # noqa: E402
from concourse import bacc, mybir  # noqa: E402
from concourse.bass_utils import run_bass_kernel_spmd  # noqa: E402

T, B, NCORES = 16384, 2048, 8
BL = 128                # time-block size (= PE contraction dim)
NB = T // BL            # 128 time blocks
C = B // NCORES         # 256 channels per core
SBW = 8                 # time blocks per superbatch
NSB = NB // SBW         # 16 superbatches
F32 = mybir.dt.float32
F16 = mybir.dt.float16
U32 = mybir.dt.uint32

TRACE = False           # set by test harness for NTFF profiling
LAST_RESULTS = None     # BassKernelResults of the last run (for profiling)

_program_cache = {}


def _reference_operator(bb, aa, n):
    """Exact linear operator of the reference filter on n samples (float64).

    Columns are responses to basis vectors; replicates the reference
    semantics: xf = zero-padded cross-correlation with b, first 5 outputs
    pass through, recurrence y[t] = xf[t] - sum_j a_j y[t-j] from t=5.
    """
    x = np.eye(n)
    xp = np.concatenate([np.zeros((4, n)), x], 0)
    xf = sum(bb[k] * xp[k:k + n] for k in range(5))
    y = xf.copy()
    at = aa[:4]
    for t in range(5, n):
        y[t] = xf[t] - (at[0] * y[t - 4] + at[1] * y[t - 3]
                        + at[2] * y[t - 2] + at[3] * y[t - 1])
    return y


def _build_mats(b32, a32):
    """Returns [BL, 3*BL] fp16: the three lhsT operands packed so the
    weights load with a single contiguous DMA (768 B per partition)."""
    bb = np.asarray(b32, np.float64)
    aa = np.asarray(a32, np.float64)
    M = _reference_operator(bb, aa, 3 * BL)
    L0 = M[0:BL, 0:BL]
    T0 = M[2 * BL:3 * BL, 2 * BL:3 * BL]
    T1 = M[2 * BL:3 * BL, BL:2 * BL]
    # truncation + init-transient leakage must be below fp32 noise
    leak = np.abs(M[2 * BL:3 * BL, 0:BL]).max()
    dev = max(np.abs(M[BL:2 * BL, BL:2 * BL] - T0).max(),
              np.abs(M[BL:2 * BL, 0:BL] - T1).max())
    assert leak < 1e-9 and dev < 1e-9, (leak, dev)

    w = np.empty((BL, 3 * BL), np.float16)
    for j, W in enumerate((L0, T0, T1)):
        w[:, j * BL:(j + 1) * BL] = W.T.astype(np.float16)  # lhsT = W.T
    return np.ascontiguousarray(w)


def _build_program():
    nc = bacc.Bacc("TRN2", target_bir_lowering=False, debug=False)
    # packed input: [s, p, {re,im}, b*C] fp16, fully contiguous per SB
    sig = nc.dram_tensor("sig", [NSB, BL, 2, SBW * C], F16,
                         kind="ExternalInput").ap()
    wd = nc.dram_tensor("w", [BL, 3 * BL], F16, kind="ExternalInput").ap()
    yd = nc.dram_tensor("y", [NSB, BL, SBW * C], F16,
                        kind="ExternalOutput").ap()

    with tile.TileContext(nc) as tc, ExitStack() as ctx:
        wpool = ctx.enter_context(tc.tile_pool(name="w", bufs=1))
        wsb = wpool.tile([BL, 3 * BL], F16, tag="w", name="w_sb")
        nc.sync.dma_start(wsb[:], wd)
        w = {"l0": wsb[:, 0:BL], "t0": wsb[:, BL:2 * BL],
             "t1": wsb[:, 2 * BL:3 * BL]}

        iopool = ctx.enter_context(tc.tile_pool(name="io", bufs=5))
        hpool = ctx.enter_context(tc.tile_pool(name="h", bufs=4))
        ypool = ctx.enter_context(tc.tile_pool(name="y", bufs=4))
        pspool = ctx.enter_context(tc.tile_pool(name="ps", bufs=2,
                                                space="PSUM"))

        def mm(ps_ap, wt, rhs_ap, start=False, stop=False):
            nc.tensor.matmul(ps_ap, w[wt], rhs_ap, start=start, stop=stop)

        prev_xh = None
        pend = {}               # s -> (ps, ysb) awaiting drain / store
        for s in range(NSB):
            pk = iopool.tile([BL, 2 * SBW * C], F16, tag="pk")
            nc.sync.dma_start(pk[:].rearrange("p (i f) -> p i f", i=2),
                              sig[s])

            # Engines are split by pipeline stage so no in-order queue
            # mixes forward work with backward (drain) work: SP issues
            # only input DMAs (prefetching as deep as the io pool
            # allows), ACT runs the squares and then the lag-2 output
            # DMA (whose drains finished last iteration, so the wait is
            # free), DVE runs the add and both PSUM drains, POOL its
            # square slice and the margin copy.
            nc.scalar.activation(pk[:, 0:10 * C], pk[:, 0:10 * C],
                                 mybir.ActivationFunctionType.Square)
            if s - 2 in pend:
                nc.scalar.dma_start(yd[s - 2], pend.pop(s - 2)[1][:])
            nc.gpsimd.tensor_mul(pk[:, 10 * C:], pk[:, 10 * C:],
                                 pk[:, 10 * C:])
            # col 0:C is a margin holding the previous superbatch's last
            # block (cross-block T1 term); uint32 bitcast halves the
            # copied element count.
            xh = hpool.tile([BL, (SBW + 1) * C], F16, tag="xh")
            nc.vector.tensor_add(xh[:, C:9 * C], pk[:, 0:8 * C],
                                 pk[:, 8 * C:16 * C])
            if s > 0:
                nc.gpsimd.tensor_copy(xh[:, 0:C].bitcast(U32),
                                      prev_xh[:, 8 * C:9 * C].bitcast(U32))

            ysb = ypool.tile([BL, SBW * C], F16, tag="ysb")
            ps = [pspool.tile([BL, 4 * C], F32, tag="ps%d" % q,
                              name="ps%d_%d" % (q, s))
                  for q in range(2)]
            # t0 terms first, then t1 (2 weight loads per superbatch,
            # overlapped by the PE reorder window). Block i of this
            # superbatch is xh[:, (1+i)*C:(2+i)*C]. Every matmul output
            # region is one whole half-bank (2C = 512 cols): one
            # accumulation group per bank, opened once, closed once.
            if s == 0:
                # block 0: exact-init operator L0, no cross term; block
                # 1 gets its own N=256 matmuls (bank-aligned).
                mm(ps[0][:, 0:C], "l0", xh[:, C:2 * C],
                   start=True, stop=True)
                mm(ps[0][:, C:2 * C], "t0", xh[:, 2 * C:3 * C], start=True)
                mm(ps[0][:, 2 * C:4 * C], "t0", xh[:, 3 * C:5 * C],
                   start=True)
                mm(ps[1][:, 0:2 * C], "t0", xh[:, 5 * C:7 * C], start=True)
                mm(ps[1][:, 2 * C:4 * C], "t0", xh[:, 7 * C:9 * C],
                   start=True)
                mm(ps[0][:, C:2 * C], "t1", xh[:, C:2 * C], stop=True)
                mm(ps[0][:, 2 * C:4 * C], "t1", xh[:, 2 * C:4 * C],
                   stop=True)
                mm(ps[1][:, 0:2 * C], "t1", xh[:, 4 * C:6 * C], stop=True)
                mm(ps[1][:, 2 * C:4 * C], "t1", xh[:, 6 * C:8 * C],
                   stop=True)
            else:
                for q in range(2):
                    o = 4 * q * C
                    mm(ps[q][:, 0:2 * C], "t0", xh[:, C + o:3 * C + o],
                       start=True)
                    mm(ps[q][:, 2 * C:4 * C], "t0",
                       xh[:, 3 * C + o:5 * C + o], start=True)
                for q in range(2):
                    o = 4 * q * C
                    mm(ps[q][:, 0:2 * C], "t1", xh[:, o:2 * C + o],
                       stop=True)
                    mm(ps[q][:, 2 * C:4 * C], "t1",
                       xh[:, 2 * C + o:4 * C + o], stop=True)

            # drains for superbatch s-1 issue at the END of the DVE/ACT
            # streams: their matmuls completed while this superbatch's
            # squares/add ran, so the waits cost nothing and never
            # block younger elementwise work.
            if s - 1 in pend:
                pps, pysb = pend[s - 1]
                nc.vector.tensor_copy(pysb[:, 0:4 * C], pps[0][:])
                nc.scalar.activation(pysb[:, 4 * C:8 * C], pps[1][:],
                                     mybir.ActivationFunctionType.Copy)
            pend[s] = (ps, ysb)
            prev_xh = xh

        # tail: drain + store the last two superbatches
        for s in (NSB - 2, NSB - 1):
            if s in pend:
                pps, pysb = pend[s]
                if s == NSB - 1:
                    nc.vector.tensor_copy(pysb[:, 0:4 * C], pps[0][:])
                    nc.scalar.activation(pysb[:, 4 * C:8 * C], pps[1][:],
                                         mybir.ActivationFunctionType.Copy)
                nc.scalar.dma_start(yd[s], pysb[:])

    nc.compile()
    return nc


def kernel(signal, b, a):
    global LAST_RESULTS
    signal = np.asarray(signal)
    assert signal.shape == (2, T, B), signal.shape

    wmat = _build_mats(np.asarray(b), np.asarray(a))

    if "prog" not in _program_cache:
        _program_cache["prog"] = _build_program()
    nc = _program_cache["prog"]

    # pack to per-core fp16 tile layout [core, s, p, {re,im}, b, c]
    pk = signal.reshape(2, NSB, SBW, BL, NCORES, C)
    pk = np.ascontiguousarray(pk.transpose(4, 1, 3, 0, 2, 5),
                              dtype=np.float16)
    pk = pk.reshape(NCORES, NSB, BL, 2, SBW * C)

    in_maps = [{"sig": pk[c], "w": wmat} for c in range(NCORES)]

    res = run_bass_kernel_spmd(nc, in_maps, core_ids=list(range(NCORES)),
                               trace=TRACE)
    LAST_RESULTS = res

    out = np.empty((T, B), np.float32)
    for c in range(NCORES):
        yc = np.asarray(res.results[c]["y"])            # [NSB, BL, SBW*C]
        yc = yc.reshape(NSB, BL, SBW, C).transpose(0, 2, 1, 3)
        out[:, c * C:(c + 1) * C] = yc.reshape(T, C).astype(np.float32)
    return out


# revision 29
# speedup vs baseline: 1.0228x; 1.0228x over previous
"""Trainium2 Bass kernel for nn_LowpassDetector.

Computes: power = re^2 + im^2, 5-tap FIR (b), order-4 IIR recurrence (a)
along time, for signal [2, T=16384, B=2048] -> y [T, B].

Strategy: the FIR+IIR cascade is LTI with all poles at radius <= 0.758,
so the combined impulse response decays below 1e-15 within 128 taps.
The whole filter is therefore exactly (to fp32) a block-Toeplitz matmul:
  y_blk[b] = T0 @ x_blk[b] + T1 @ x_blk[b-1]     (b >= 1)
  y_blk[0] = L0 @ x_blk[0]
with L0 the exact 128x128 operator of the reference recurrence
(including its "first 5 samples pass through" initial condition), built
on the host in float64. Channels (2048) are sharded 256 per core across
8 cores; time blocks of 128 map to the TensorEngine contraction dim.

v6 design (what measurement drove what):
- v3 was DMA-bound at the fp32 roofline (50.5 MB/core at ~310 GB/s).
  The rel-err budget (2e-2) has huge headroom (v3: 3.6e-4), so all I/O
  is fp16: host pre-packs the signal into the per-superbatch SBUF tile
  layout [NSB, 128, 2{re,im}, SBW*C] fp16 (one contiguous 1 MB DMA per
  superbatch, 8 KB/partition lines); y returns as fp16 tiles and is
  cast/scattered on the host. 24 MB/core total.
- Per-packet DMA rate is at line rate (26.5 GB/s/engine) but engines
  idled ~40% in multi-us gaps: pipeline stalls, not transfer speed.
  Root cause was head-of-line blocking in the in-order engine queues
  (e.g. DVE's PSUM drain for superbatch s waits on PE mid-queue and
  blocks superbatch s+1's square behind it). So the issue order is
  software-pipelined: drains and the output DMA of superbatch s-1 are
  issued at the top of iteration s; no instruction ever waits on a
  dependency from its own superbatch while blocking the next one.
- Engine balance per superbatch (~4.4 us DMA pace): ACT = 9C square +
  ps1 drain; DVE = 3C square (not in-place: in-place tensor_mul missed
  the fp16 2x path) + power add (fp16 2x) + ps0 drain + margin copy
  (bitcast to uint32 to halve element count); POOL (no PSUM port) =
  4C square. PE = 8 matmuls (t0 grouped then t1, 2 weight loads).
- PSUM rules learned the hard way: a matmul output region must not
  straddle a 2 KB bank boundary, and within one bank accumulation
  groups must not interleave (start=True clears the whole bank's
  has_written bits) -- each half-bank holds one group, opened once,
  closed once; the margin copy keeps block 0's T1 matmul in the same
  group shape as blocks 1-3.
"""

import sys
from contextlib import ExitStack

import numpy as np

for _p in ("/opt/trn_rl_repo",):
    if _p not in sys.path:
        sys.path.insert(0, _p)

import concourse.bass as bass  # noqa: E402
import concourse.tile as tile  # noqa: E402
from concourse import bacc, mybir  # noqa: E402
from concourse.bass_utils import run_bass_kernel_spmd  # noqa: E402

T, B, NCORES = 16384, 2048, 8
BL = 128                # time-block size (= PE contraction dim)
NB = T // BL            # 128 time blocks
C = B // NCORES         # 256 channels per core
SBW = 8                 # time blocks per superbatch
NSB = NB // SBW         # 16 superbatches
F32 = mybir.dt.float32
F16 = mybir.dt.float16
U32 = mybir.dt.uint32

TRACE = False           # set by test harness for NTFF profiling
LAST_RESULTS = None     # BassKernelResults of the last run (for profiling)

_program_cache = {}


def _reference_operator(bb, aa, n):
    """Exact linear operator of the reference filter on n samples (float64).

    Columns are responses to basis vectors; replicates the reference
    semantics: xf = zero-padded cross-correlation with b, first 5 outputs
    pass through, recurrence y[t] = xf[t] - sum_j a_j y[t-j] from t=5.
    """
    x = np.eye(n)
    xp = np.concatenate([np.zeros((4, n)), x], 0)
    xf = sum(bb[k] * xp[k:k + n] for k in range(5))
    y = xf.copy()
    at = aa[:4]
    for t in range(5, n):
        y[t] = xf[t] - (at[0] * y[t - 4] + at[1] * y[t - 3]
                        + at[2] * y[t - 2] + at[3] * y[t - 1])
    return y


def _build_mats(b32, a32):
    """Returns [BL, 3*BL] fp16: the three lhsT operands packed so the
    weights load with a single contiguous DMA (768 B per partition)."""
    bb = np.asarray(b32, np.float64)
    aa = np.asarray(a32, np.float64)
    M = _reference_operator(bb, aa, 3 * BL)
    L0 = M[0:BL, 0:BL]
    T0 = M[2 * BL:3 * BL, 2 * BL:3 * BL]
    T1 = M[2 * BL:3 * BL, BL:2 * BL]
    # truncation + init-transient leakage must be below fp32 noise
    leak = np.abs(M[2 * BL:3 * BL, 0:BL]).max()
    dev = max(np.abs(M[BL:2 * BL, BL:2 * BL] - T0).max(),
              np.abs(M[BL:2 * BL, 0:BL] - T1).max())
    assert leak < 1e-9 and dev < 1e-9, (leak, dev)

    w = np.empty((BL, 3 * BL), np.float16)
    for j, W in enumerate((L0, T0, T1)):
        w[:, j * BL:(j + 1) * BL] = W.T.astype(np.float16)  # lhsT = W.T
    return np.ascontiguousarray(w)


def _build_program():
    nc = bacc.Bacc("TRN2", target_bir_lowering=False, debug=False)
    # packed input: [s, p, {re,im}, b*C] fp16, fully contiguous per SB
    sig = nc.dram_tensor("sig", [NSB, BL, 2, SBW * C], F16,
                         kind="ExternalInput").ap()
    wd = nc.dram_tensor("w", [BL, 3 * BL], F16, kind="ExternalInput").ap()
    yd = nc.dram_tensor("y", [NSB, BL, SBW * C], F16,
                        kind="ExternalOutput").ap()

    with tile.TileContext(nc) as tc, ExitStack() as ctx:
        wpool = ctx.enter_context(tc.tile_pool(name="w", bufs=1))
        wsb = wpool.tile([BL, 3 * BL], F16, tag="w", name="w_sb")
        nc.sync.dma_start(wsb[:], wd)
        w = {"l0": wsb[:, 0:BL], "t0": wsb[:, BL:2 * BL],
             "t1": wsb[:, 2 * BL:3 * BL]}

        iopool = ctx.enter_context(tc.tile_pool(name="io", bufs=5))
        hpool = ctx.enter_context(tc.tile_pool(name="h", bufs=4))
        ypool = ctx.enter_context(tc.tile_pool(name="y", bufs=5))
        pspool = ctx.enter_context(tc.tile_pool(name="ps", bufs=2,
                                                space="PSUM"))

        def mm(ps_ap, wt, rhs_ap, start=False, stop=False):
            nc.tensor.matmul(ps_ap, w[wt], rhs_ap, start=start, stop=stop)

        prev_xh = None
        pend = {}               # s -> (ps, ysb) awaiting drain / store
        for s in range(NSB):
            pk = iopool.tile([BL, 2 * SBW * C], F16, tag="pk")
            nc.sync.dma_start(pk[:].rearrange("p (i f) -> p i f", i=2),
                              sig[s])

            # Engines are split by pipeline stage so no in-order queue
            # mixes forward work with backward (drain) work: SP issues
            # only input DMAs (prefetching as deep as the io pool
            # allows), ACT runs the squares and then the lag-2 output
            # DMA (whose drains finished last iteration, so the wait is
            # free), DVE runs the add and both PSUM drains, POOL its
            # square slice and the margin copy.
            nc.scalar.activation(pk[:, 0:12 * C], pk[:, 0:12 * C],
                                 mybir.ActivationFunctionType.Square)
            if s - 3 in pend:
                nc.scalar.dma_start(yd[s - 3], pend.pop(s - 3)[1][:])
            nc.gpsimd.tensor_mul(pk[:, 12 * C:], pk[:, 12 * C:],
                                 pk[:, 12 * C:])
            # col 0:C is a margin holding the previous superbatch's last
            # block (cross-block T1 term); uint32 bitcast halves the
            # copied element count.
            xh = hpool.tile([BL, (SBW + 1) * C], F16, tag="xh")
            nc.vector.tensor_add(xh[:, C:9 * C], pk[:, 0:8 * C],
                                 pk[:, 8 * C:16 * C])
            if s > 0:
                nc.gpsimd.tensor_copy(xh[:, 0:C].bitcast(U32),
                                      prev_xh[:, 8 * C:9 * C].bitcast(U32))

            ysb = ypool.tile([BL, SBW * C], F16, tag="ysb")
            ps = [pspool.tile([BL, 4 * C], F32, tag="ps%d" % q,
                              name="ps%d_%d" % (q, s))
                  for q in range(2)]
            # t0 terms first, then t1 (2 weight loads per superbatch,
            # overlapped by the PE reorder window). Block i of this
            # superbatch is xh[:, (1+i)*C:(2+i)*C]. Every matmul output
            # region is one whole half-bank (2C = 512 cols): one
            # accumulation group per bank, opened once, closed once.
            if s == 0:
                # block 0: exact-init operator L0, no cross term; block
                # 1 gets its own N=256 matmuls (bank-aligned).
                mm(ps[0][:, 0:C], "l0", xh[:, C:2 * C],
                   start=True, stop=True)
                mm(ps[0][:, C:2 * C], "t0", xh[:, 2 * C:3 * C], start=True)
                mm(ps[0][:, 2 * C:4 * C], "t0", xh[:, 3 * C:5 * C],
                   start=True)
                mm(ps[1][:, 0:2 * C], "t0", xh[:, 5 * C:7 * C], start=True)
                mm(ps[1][:, 2 * C:4 * C], "t0", xh[:, 7 * C:9 * C],
                   start=True)
                mm(ps[0][:, C:2 * C], "t1", xh[:, C:2 * C], stop=True)
                mm(ps[0][:, 2 * C:4 * C], "t1", xh[:, 2 * C:4 * C],
                   stop=True)
                mm(ps[1][:, 0:2 * C], "t1", xh[:, 4 * C:6 * C], stop=True)
                mm(ps[1][:, 2 * C:4 * C], "t1", xh[:, 6 * C:8 * C],
                   stop=True)
            else:
                for q in range(2):
                    o = 4 * q * C
                    mm(ps[q][:, 0:2 * C], "t0", xh[:, C + o:3 * C + o],
                       start=True)
                    mm(ps[q][:, 2 * C:4 * C], "t0",
                       xh[:, 3 * C + o:5 * C + o], start=True)
                for q in range(2):
                    o = 4 * q * C
                    mm(ps[q][:, 0:2 * C], "t1", xh[:, o:2 * C + o],
                       stop=True)
                    mm(ps[q][:, 2 * C:4 * C], "t1",
                       xh[:, 2 * C + o:4 * C + o], stop=True)

            # drains for superbatch s-1 issue at the END of the DVE/ACT
            # streams: their matmuls completed while this superbatch's
            # squares/add ran, so the waits cost nothing and never
            # block younger elementwise work.
            if s - 1 in pend:
                pps, pysb = pend[s - 1]
                nc.vector.tensor_copy(pysb[:, 0:4 * C], pps[0][:])
                nc.vector.tensor_copy(pysb[:, 4 * C:8 * C], pps[1][:])
            pend[s] = (ps, ysb)
            prev_xh = xh

        # tail: drain + store the last three superbatches
        for s in (NSB - 3, NSB - 2, NSB - 1):
            if s in pend:
                pps, pysb = pend[s]
                if s == NSB - 1:
                    nc.vector.tensor_copy(pysb[:, 0:4 * C], pps[0][:])
                    nc.vector.tensor_copy(pysb[:, 4 * C:8 * C], pps[1][:])
                nc.scalar.dma_start(yd[s], pysb[:])

    nc.compile()
    return nc


def kernel(signal, b, a):
    global LAST_RESULTS
    signal = np.asarray(signal)
    assert signal.shape == (2, T, B), signal.shape

    wmat = _build_mats(np.asarray(b), np.asarray(a))

    if "prog" not in _program_cache:
        _program_cache["prog"] = _build_program()
    nc = _program_cache["prog"]

    # pack to per-core fp16 tile layout [core, s, p, {re,im}, b, c]
    pk = signal.reshape(2, NSB, SBW, BL, NCORES, C)
    pk = np.ascontiguousarray(pk.transpose(4, 1, 3, 0, 2, 5),
                              dtype=np.float16)
    pk = pk.reshape(NCORES, NSB, BL, 2, SBW * C)

    in_maps = [{"sig": pk[c], "w": wmat} for c in range(NCORES)]

    res = run_bass_kernel_spmd(nc, in_maps, core_ids=list(range(NCORES)),
                               trace=TRACE)
    LAST_RESULTS = res

    out = np.empty((T, B), np.float32)
    for c in range(NCORES):
        yc = np.asarray(res.results[c]["y"])            # [NSB, BL, SBW*C]
        yc = yc.reshape(NSB, BL, SBW, C).transpose(0, 2, 1, 3)
        out[:, c * C:(c + 1) * C] = yc.reshape(T, C).astype(np.float32)
    return out


# revision 30
# speedup vs baseline: 1.0487x; 1.0254x over previous
"""Trainium2 Bass kernel for nn_LowpassDetector.

Computes: power = re^2 + im^2, 5-tap FIR (b), order-4 IIR recurrence (a)
along time, for signal [2, T=16384, B=2048] -> y [T, B].

Strategy: the FIR+IIR cascade is LTI with all poles at radius <= 0.758,
so the combined impulse response decays below 1e-15 within 128 taps.
The whole filter is therefore exactly (to fp32) a block-Toeplitz matmul:
  y_blk[b] = T0 @ x_blk[b] + T1 @ x_blk[b-1]     (b >= 1)
  y_blk[0] = L0 @ x_blk[0]
with L0 the exact 128x128 operator of the reference recurrence
(including its "first 5 samples pass through" initial condition), built
on the host in float64. Channels (2048) are sharded 256 per core across
8 cores; time blocks of 128 map to the TensorEngine contraction dim.

v6 design (what measurement drove what):
- v3 was DMA-bound at the fp32 roofline (50.5 MB/core at ~310 GB/s).
  The rel-err budget (2e-2) has huge headroom (v3: 3.6e-4), so all I/O
  is fp16: host pre-packs the signal into the per-superbatch SBUF tile
  layout [NSB, 128, 2{re,im}, SBW*C] fp16 (one contiguous 1 MB DMA per
  superbatch, 8 KB/partition lines); y returns as fp16 tiles and is
  cast/scattered on the host. 24 MB/core total.
- Per-packet DMA rate is at line rate (26.5 GB/s/engine) but engines
  idled ~40% in multi-us gaps: pipeline stalls, not transfer speed.
  Root cause was head-of-line blocking in the in-order engine queues
  (e.g. DVE's PSUM drain for superbatch s waits on PE mid-queue and
  blocks superbatch s+1's square behind it). So the issue order is
  software-pipelined: drains and the output DMA of superbatch s-1 are
  issued at the top of iteration s; no instruction ever waits on a
  dependency from its own superbatch while blocking the next one.
- Engine balance per superbatch (~4.4 us DMA pace): ACT = 9C square +
  ps1 drain; DVE = 3C square (not in-place: in-place tensor_mul missed
  the fp16 2x path) + power add (fp16 2x) + ps0 drain + margin copy
  (bitcast to uint32 to halve element count); POOL (no PSUM port) =
  4C square. PE = 8 matmuls (t0 grouped then t1, 2 weight loads).
- PSUM rules learned the hard way: a matmul output region must not
  straddle a 2 KB bank boundary, and within one bank accumulation
  groups must not interleave (start=True clears the whole bank's
  has_written bits) -- each half-bank holds one group, opened once,
  closed once; the margin copy keeps block 0's T1 matmul in the same
  group shape as blocks 1-3.
"""

import sys
from contextlib import ExitStack

import numpy as np

for _p in ("/opt/trn_rl_repo",):
    if _p not in sys.path:
        sys.path.insert(0, _p)

import concourse.bass as bass  # noqa: E402
import concourse.tile as tile  # noqa: E402
from concourse import bacc, mybir  # noqa: E402
from concourse.bass_utils import run_bass_kernel_spmd  # noqa: E402

T, B, NCORES = 16384, 2048, 8
BL = 128                # time-block size (= PE contraction dim)
NB = T // BL            # 128 time blocks
C = B // NCORES         # 256 channels per core
SBW = 8                 # time blocks per superbatch
NSB = NB // SBW         # 16 superbatches
F32 = mybir.dt.float32
F16 = mybir.dt.float16
U32 = mybir.dt.uint32

TRACE = False           # set by test harness for NTFF profiling
LAST_RESULTS = None     # BassKernelResults of the last run (for profiling)

_program_cache = {}


def _reference_operator(bb, aa, n):
    """Exact linear operator of the reference filter on n samples (float64).

    Columns are responses to basis vectors; replicates the reference
    semantics: xf = zero-padded cross-correlation with b, first 5 outputs
    pass through, recurrence y[t] = xf[t] - sum_j a_j y[t-j] from t=5.
    """
    x = np.eye(n)
    xp = np.concatenate([np.zeros((4, n)), x], 0)
    xf = sum(bb[k] * xp[k:k + n] for k in range(5))
    y = xf.copy()
    at = aa[:4]
    for t in range(5, n):
        y[t] = xf[t] - (at[0] * y[t - 4] + at[1] * y[t - 3]
                        + at[2] * y[t - 2] + at[3] * y[t - 1])
    return y


def _build_mats(b32, a32):
    """Returns [BL, 3*BL] fp16: the three lhsT operands packed so the
    weights load with a single contiguous DMA (768 B per partition)."""
    bb = np.asarray(b32, np.float64)
    aa = np.asarray(a32, np.float64)
    M = _reference_operator(bb, aa, 3 * BL)
    L0 = M[0:BL, 0:BL]
    T0 = M[2 * BL:3 * BL, 2 * BL:3 * BL]
    T1 = M[2 * BL:3 * BL, BL:2 * BL]
    # truncation + init-transient leakage must be below fp32 noise
    leak = np.abs(M[2 * BL:3 * BL, 0:BL]).max()
    dev = max(np.abs(M[BL:2 * BL, BL:2 * BL] - T0).max(),
              np.abs(M[BL:2 * BL, 0:BL] - T1).max())
    assert leak < 1e-9 and dev < 1e-9, (leak, dev)

    w = np.empty((BL, 3 * BL), np.float16)
    for j, W in enumerate((L0, T0, T1)):
        w[:, j * BL:(j + 1) * BL] = W.T.astype(np.float16)  # lhsT = W.T
    return np.ascontiguousarray(w)


def _build_program():
    nc = bacc.Bacc("TRN2", target_bir_lowering=False, debug=False)
    # packed input: [s, p, {re,im}, b*C] fp16, fully contiguous per SB
    sig = nc.dram_tensor("sig", [NSB, BL, 2, SBW * C], F16,
                         kind="ExternalInput").ap()
    wd = nc.dram_tensor("w", [BL, 3 * BL], F16, kind="ExternalInput").ap()
    yd = nc.dram_tensor("y", [NSB, BL, SBW * C], F16,
                        kind="ExternalOutput").ap()

    with tile.TileContext(nc) as tc, ExitStack() as ctx:
        wpool = ctx.enter_context(tc.tile_pool(name="w", bufs=1))
        wsb = wpool.tile([BL, 3 * BL], F16, tag="w", name="w_sb")
        nc.sync.dma_start(wsb[:], wd)
        w = {"l0": wsb[:, 0:BL], "t0": wsb[:, BL:2 * BL],
             "t1": wsb[:, 2 * BL:3 * BL]}

        iopool = ctx.enter_context(tc.tile_pool(name="io", bufs=5))
        hpool = ctx.enter_context(tc.tile_pool(name="h", bufs=4))
        ypool = ctx.enter_context(tc.tile_pool(name="y", bufs=5))
        pspool = ctx.enter_context(tc.tile_pool(name="ps", bufs=2,
                                                space="PSUM"))

        def mm(ps_ap, wt, rhs_ap, start=False, stop=False):
            nc.tensor.matmul(ps_ap, w[wt], rhs_ap, start=start, stop=stop)

        prev_xh = None
        pend = {}               # s -> (ps, ysb) awaiting drain / store
        for s in range(NSB):
            pk = iopool.tile([BL, 2 * SBW * C], F16, tag="pk")
            nc.sync.dma_start(pk[:].rearrange("p (i f) -> p i f", i=2),
                              sig[s])

            # Engines are split by pipeline stage so no in-order queue
            # mixes forward work with backward (drain) work: SP issues
            # only input DMAs (prefetching as deep as the io pool
            # allows), ACT runs the squares and then the lag-2 output
            # DMA (whose drains finished last iteration, so the wait is
            # free), DVE runs the add and both PSUM drains, POOL its
            # square slice and the margin copy.
            nc.scalar.activation(pk[:, 0:12 * C], pk[:, 0:12 * C],
                                 mybir.ActivationFunctionType.Square)
            if s - 2 in pend:
                nc.scalar.dma_start(yd[s - 2], pend.pop(s - 2)[1][:])
            nc.gpsimd.tensor_mul(pk[:, 12 * C:], pk[:, 12 * C:],
                                 pk[:, 12 * C:])
            # col 0:C is a margin holding the previous superbatch's last
            # block (cross-block T1 term); uint32 bitcast halves the
            # copied element count.
            xh = hpool.tile([BL, (SBW + 1) * C], F16, tag="xh")
            nc.vector.tensor_add(xh[:, C:9 * C], pk[:, 0:8 * C],
                                 pk[:, 8 * C:16 * C])
            if s > 0:
                nc.gpsimd.tensor_copy(xh[:, 0:C].bitcast(U32),
                                      prev_xh[:, 8 * C:9 * C].bitcast(U32))

            ysb = ypool.tile([BL, SBW * C], F16, tag="ysb")
            ps = [pspool.tile([BL, 4 * C], F32, tag="ps%d" % q,
                              name="ps%d_%d" % (q, s))
                  for q in range(2)]
            # t0 terms first, then t1 (2 weight loads per superbatch,
            # overlapped by the PE reorder window). Block i of this
            # superbatch is xh[:, (1+i)*C:(2+i)*C]. Every matmul output
            # region is one whole half-bank (2C = 512 cols): one
            # accumulation group per bank, opened once, closed once.
            if s == 0:
                # block 0: exact-init operator L0, no cross term; block
                # 1 gets its own N=256 matmuls (bank-aligned).
                mm(ps[0][:, 0:C], "l0", xh[:, C:2 * C],
                   start=True, stop=True)
                mm(ps[0][:, C:2 * C], "t0", xh[:, 2 * C:3 * C], start=True)
                mm(ps[0][:, 2 * C:4 * C], "t0", xh[:, 3 * C:5 * C],
                   start=True)
                mm(ps[1][:, 0:2 * C], "t0", xh[:, 5 * C:7 * C], start=True)
                mm(ps[1][:, 2 * C:4 * C], "t0", xh[:, 7 * C:9 * C],
                   start=True)
                mm(ps[0][:, C:2 * C], "t1", xh[:, C:2 * C], stop=True)
                mm(ps[0][:, 2 * C:4 * C], "t1", xh[:, 2 * C:4 * C],
                   stop=True)
                mm(ps[1][:, 0:2 * C], "t1", xh[:, 4 * C:6 * C], stop=True)
                mm(ps[1][:, 2 * C:4 * C], "t1", xh[:, 6 * C:8 * C],
                   stop=True)
            else:
                for q in range(2):
                    o = 4 * q * C
                    mm(ps[q][:, 0:2 * C], "t0", xh[:, C + o:3 * C + o],
                       start=True)
                    mm(ps[q][:, 2 * C:4 * C], "t0",
                       xh[:, 3 * C + o:5 * C + o], start=True)
                for q in range(2):
                    o = 4 * q * C
                    mm(ps[q][:, 0:2 * C], "t1", xh[:, o:2 * C + o],
                       stop=True)
                    mm(ps[q][:, 2 * C:4 * C], "t1",
                       xh[:, 2 * C + o:4 * C + o], stop=True)

            # drains for superbatch s-1 issue at the END of the DVE/ACT
            # streams: their matmuls completed while this superbatch's
            # squares/add ran, so the waits cost nothing and never
            # block younger elementwise work.
            if s - 1 in pend:
                pps, pysb = pend[s - 1]
                nc.vector.tensor_copy(pysb[:, 0:4 * C], pps[0][:])
                nc.vector.tensor_copy(pysb[:, 4 * C:8 * C], pps[1][:])
            pend[s] = (ps, ysb)
            prev_xh = xh

        # tail: drain + store the last three superbatches
        for s in (NSB - 2, NSB - 1):
            if s in pend:
                pps, pysb = pend[s]
                if s == NSB - 1:
                    nc.vector.tensor_copy(pysb[:, 0:4 * C], pps[0][:])
                    nc.vector.tensor_copy(pysb[:, 4 * C:8 * C], pps[1][:])
                nc.scalar.dma_start(yd[s], pysb[:])

    nc.compile()
    return nc


def kernel(signal, b, a):
    global LAST_RESULTS
    signal = np.asarray(signal)
    assert signal.shape == (2, T, B), signal.shape

    wmat = _build_mats(np.asarray(b), np.asarray(a))

    if "prog" not in _program_cache:
        _program_cache["prog"] = _build_program()
    nc = _program_cache["prog"]

    # pack to per-core fp16 tile layout [core, s, p, {re,im}, b, c]
    pk = signal.reshape(2, NSB, SBW, BL, NCORES, C)
    pk = np.ascontiguousarray(pk.transpose(4, 1, 3, 0, 2, 5),
                              dtype=np.float16)
    pk = pk.reshape(NCORES, NSB, BL, 2, SBW * C)

    in_maps = [{"sig": pk[c], "w": wmat} for c in range(NCORES)]

    res = run_bass_kernel_spmd(nc, in_maps, core_ids=list(range(NCORES)),
                               trace=TRACE)
    LAST_RESULTS = res

    out = np.empty((T, B), np.float32)
    for c in range(NCORES):
        yc = np.asarray(res.results[c]["y"])            # [NSB, BL, SBW*C]
        yc = yc.reshape(NSB, BL, SBW, C).transpose(0, 2, 1, 3)
        out[:, c * C:(c + 1) * C] = yc.reshape(T, C).astype(np.float32)
    return out
